# revision 1
# baseline (speedup 1.0000x reference)
"""Multi-head self-attention (B=2, T=2048, C=1024, H=16, RoPE, causal) on 8 trn2 cores.

v2: fused bf16 pipeline.
  - Sharding: data-parallel over batch (2) x tensor-parallel over head groups (4).
    Core c handles batch c//4, heads (c%4)*4 .. +3; host sums 4 partials/batch.
  - All inputs host-cast to bf16 (rel err ~6e-3 << 2e-2 tolerance); y partials
    are written bf16 and summed f32 on the host.
  - x^T tiles built by DMA-XBAR transpose straight from DRAM (no PE transposes).
  - era1 (projections+RoPE) and era2 (attention) fused per 512-row quarter:
    attention for query-chunk qc consumes exactly the k/v tiles of quarters
    <= qc, so exp/DVE/Pool work overlaps the projection matmuls.  The
    out-projection of chunk qc is deferred until after era1(qtr+1) so the PE
    never head-of-line blocks on the normalize chain.
  - RoPE writes rotated q^T/k^T directly into the per-head-contiguous tiles
    with partition-offset DVE ops (no SBUF->SBUF merge DMAs).
  - Engine split: PE matmuls; Act pure exp (no table churn); DVE RoPE +
    proj evictions + reciprocal + normalize; Pool v-evict, tri-mask, y-evict.
"""
import sys
import math

sys.path.insert(0, "/opt/trn_rl_repo")

import numpy as np

B, T, C, H, D = 2, 2048, 1024, 16, 64
NCORES = 8
NKC = C // 128         # 8 contraction chunks
NQTR = T // 512        # 4 t-quarters
NKT = T // 128         # 16 k-tiles
ROPE_BASE = 10000.0

_BUILT = None


# ---------------------------------------------------------------------------
# Toolchain workaround: this walrus build accepts at most ONE semaphore wait
# per instruction.  (a) replace Tile's exit drain with a chain of single-wait
# drains; (b) hoist extra waits onto same-engine nops.
# ---------------------------------------------------------------------------

def _apply_tile_patch():
    import bass_rust
    import concourse.tile as tile
    from concourse.vector_clock import ScopedClock

    def _patched_drain_and_barrier(self, tick_clock, wait_clock):
        nc = self.nc
        probe = nc.sync.drain()
        wait_clock.add_sem_waits(probe.ins, ScopedClock({None: tick_clock.global_clock}))
        si = probe.ins.sync_info
        waits = list(si.on_wait) if si is not None else []
        probe.ins.sync_info = None
        name2sem = {s.name: s for s in wait_clock.sems.allocated().values()}
        for w in waits:
            d = nc.sync.drain()
            bass_rust.wait_op(d.ins, name2sem[w.ant_name], w.wait_value, "sem-ge", False)
        nc.all_engine_barrier()
        popped = nc._tile_sem_poison_stack.pop()
        assert popped is self._sem_poison
        nc.clear_and_free_semaphores(list(self.sems.allocated().values()))
        nc.all_engine_barrier()

    tile.TileContext._drain_and_barrier = _patched_drain_and_barrier


def _split_multi_waits(nc):
    import bass_rust
    import concourse.mybir as mybir

    ctr = 0
    for fn in nc.m.functions:
        for blk in fn.blocks:
            il = blk.instructions
            new = []
            changed = False
            for inst in il:
                si = inst.sync_info
                waits = list(si.on_wait) if si is not None else []
                if len(waits) > 1:
                    changed = True
                    for w in waits[:-1]:
                        nop = mybir.InstNoOp(name=f"I-waitsplit-{ctr}", ins=[], outs=[])
                        ctr += 1
                        nop.engine = inst.engine
                        nop.sync_info = bass_rust.SyncInfo(on_wait=[w], on_update=[])
                        new.append(nop)
                    inst.sync_info = bass_rust.SyncInfo(
                        on_wait=[waits[-1]], on_update=list(si.on_update)
                    )
                new.append(inst)
            if changed:
                blk.instructions = new


# ---------------------------------------------------------------------------
# Kernel builder (per-core program; identical on all 8 cores)
# ---------------------------------------------------------------------------

def build_nc(split_waits=True, loop_iters=None, phases=(1, 2), unroll=None):
    if unroll is None:
        unroll = 2 if loop_iters else 1
    if loop_iters:
        assert loop_iters % unroll == 0
        loop_iters = loop_iters // unroll
    _apply_tile_patch()
    import concourse.bass as bass
    import concourse.tile as tile
    import concourse.mybir as mybir
    from contextlib import nullcontext

    dt = mybir.dt
    f32, bf16 = dt.float32, dt.bfloat16
    Exp = mybir.ActivationFunctionType.Exp
    MUL, SUB, ADD = (mybir.AluOpType.mult, mybir.AluOpType.subtract,
                     mybir.AluOpType.add)

    nc = bass.Bass()
    x_d = nc.dram_tensor("x", [T, C], bf16, kind="ExternalInput")
    w_d = nc.dram_tensor("w", [C, 768], bf16, kind="ExternalInput")
    wo_d = nc.dram_tensor("wo", [256, C], bf16, kind="ExternalInput")
    cs_d = nc.dram_tensor("cs", [128, T], bf16, kind="ExternalInput")
    sn_d = nc.dram_tensor("sn", [128, T], bf16, kind="ExternalInput")
    # tri is the additive causal mask for S^T diagonal blocks (0 on/below
    # diagonal, -240 above); id is a 128x128 identity for the mask matmul
    tri_d = nc.dram_tensor("tri", [128, 128], bf16, kind="ExternalInput")
    id_d = nc.dram_tensor("id", [128, 128], bf16, kind="ExternalInput")
    y_d = nc.dram_tensor("y", [T, C], bf16, kind="ExternalOutput")

    with tile.TileContext(nc) as tc:
      loop_cm = (tc.For_i(0, loop_iters, 1,
                          hint_engines=(mybir.EngineType.PE, mybir.EngineType.Activation,
                                        mybir.EngineType.DVE, mybir.EngineType.SP,
                                        mybir.EngineType.Pool))
                 if loop_iters else nullcontext())
      with loop_cm:
        with (
            tc.tile_pool(name="persist", bufs=1) as persist,
            tc.tile_pool(name="vsb", bufs=2) as vsb_pool,
            tc.tile_pool(name="qkT", bufs=2) as qkT_pool,
            tc.tile_pool(name="asb", bufs=2) as asb_pool,
            tc.tile_pool(name="w", bufs=1) as w_pool,
            tc.tile_pool(name="xT", bufs=2) as xT_pool,
            tc.tile_pool(name="rope", bufs=2) as rope_pool,
            tc.tile_pool(name="pt", bufs=3) as pt_pool,
            tc.tile_pool(name="nrm", bufs=2) as nrm_pool,
            tc.tile_pool(name="yout", bufs=2) as y_pool,
            tc.tile_pool(name="pconst", bufs=1) as pconst,
            tc.tile_pool(name="ps_a", bufs=1, space="PSUM") as ps_a_pool,
                        tc.tile_pool(name="ps_s", bufs=2, space="PSUM") as ps_s_pool,
            tc.tile_pool(name="ps_o", bufs=2, space="PSUM") as ps_o_pool,
        ):
          def iter_body(u):
            # per-head-contiguous rotated q^T/k^T: tile [128, T] = 2 heads,
            # rows [h_ev(32); h_od(32); h'_ev(32); h'_od(32)].  bufs=2 pools
            # alternate buffers between the two unrolled bodies so iteration
            # i+1's era1 overlaps iteration i's era2.
            qT = [qkT_pool.tile([128, T], bf16, tag=f"qT{i}", name=f"qT{i}_{u}") for i in range(2)]
            kT = [qkT_pool.tile([128, T], bf16, tag=f"kT{i}", name=f"kT{i}_{u}") for i in range(2)]
            # v in (t, d) layout + ones column per head slot: [128, kt, 4*65]
            v_sb = vsb_pool.tile([128, NKT, 4 * 65], bf16, tag="v")
            # PE-consumed constants double-buffered: their per-body reload
            # must not WAR-block on the previous body's tail reads
            wo_sb = pconst.tile([128, 2, C], bf16, tag="wo")
            tri_sb = pconst.tile([128, 128], bf16, tag="tri")
            id_sb = pconst.tile([128, 128], bf16, tag="id")
            cs_sb = persist.tile([128, T], bf16, tag="cs")
            sn_sb = persist.tile([128, T], bf16, tag="sn")
            w_sb = w_pool.tile([128, NKC, 768], bf16, tag="w")
            a_sb = [asb_pool.tile([128, T], bf16, tag=f"a{i}", name=f"a{i}_{u}") for i in range(2)]

            # ones columns of v (col 64 of each 65-wide head slot)
            v4 = v_sb[:].rearrange("p kt (h c) -> p kt h c", h=4)
            nc.gpsimd.memset(v4[:, :, :, 64:65], 1.0)

            def era1(qtr):
                xT_q = xT_pool.tile([128, NKC, 512], bf16, tag="xTq")
                for kc in range(NKC):
                    if qtr == 0 and kc % 2 == 0:
                        # interleave w chunks with the x transposes so the
                        # first projection matmuls can start early
                        nc.sync.dma_start(
                            w_sb[:, kc:kc + 2, :],
                            w_d[kc * 128:(kc + 2) * 128, :].rearrange("(kc p) f -> p kc f", p=128))
                    nc.sync.dma_start_transpose(
                        xT_q[:, kc, :],
                        x_d[qtr * 512:(qtr + 1) * 512, kc * 128:(kc + 1) * 128])
                if qtr == 0:
                    nc.sync.dma_start(cs_sb[:], cs_d[:])
                    nc.sync.dma_start(sn_sb[:], sn_d[:])
                    nc.sync.dma_start(tri_sb[:], tri_d[:])
                    nc.sync.dma_start(id_sb[:], id_d[:])
                    nc.sync.dma_start(wo_sb[:], wo_d[:].rearrange("(kc p) c -> p kc c", p=128))

                # ---- QK projection + RoPE (pairs: (QE,QO) then (KE,KO)),
                # V-projection tiles interleaved to cover eviction latency
                cs_c = cs_sb[:, qtr * 512:(qtr + 1) * 512]
                sn_c = sn_sb[:, qtr * 512:(qtr + 1) * 512]
                sl = slice(qtr * 512, (qtr + 1) * 512)

                def v_proj(tl):
                    psv = ps_s_pool.tile([128, 2, 512], f32, tag="s")
                    for kc in range(NKC):
                        nc.tensor.matmul(psv[:, 0, 0:256], xT_q[:, kc, tl * 128:(tl + 1) * 128],
                                         w_sb[:, kc, 512:768], start=(kc == 0), stop=(kc == NKC - 1),
                                         skip_group_check=True)
                    kt = qtr * 4 + tl
                    # Act-engine copy: 'copy' shares the exp activation table,
                    # so no table reloads alternate with era2's exps
                    nc.scalar.copy(v4[:, kt, :, 0:64],
                                   psv[:, 0, 0:256].rearrange("p (h d) -> p h d", h=4))

                for pair in range(2):          # 0: Q, 1: K
                    m_e, m_o = 2 * pair, 2 * pair + 1
                    ps_eo = ps_a_pool.tile([128, 2, 512], f32, tag="proj")
                    for kc in range(NKC):
                        nc.tensor.matmul(ps_eo[:, 0, :], w_sb[:, kc, m_e * 128:(m_e + 1) * 128],
                                         xT_q[:, kc, :], start=(kc == 0), stop=(kc == NKC - 1),
                                         skip_group_check=True)
                        nc.tensor.matmul(ps_eo[:, 1, :], w_sb[:, kc, m_o * 128:(m_o + 1) * 128],
                                         xT_q[:, kc, :], start=(kc == 0), stop=(kc == NKC - 1),
                                         skip_group_check=True)
                    # PSUM eviction in one shot; rope math on DVE (2-byte 2x
                    # mode, ~3.4x faster than GPSIMD's software TT).  At the
                    # body boundary (qtr 0) Act is still draining the previous
                    # body's exp backlog, so evict on DVE there instead.
                    eo_sb = rope_pool.tile([128, 2, 512], bf16, tag="eo")
                    if qtr == 0:
                        nc.vector.tensor_copy(eo_sb[:].rearrange("p a b -> p (a b)"),
                                              ps_eo[:].rearrange("p a b -> p (a b)"))
                    else:
                        nc.scalar.copy(eo_sb[:].rearrange("p a b -> p (a b)"),
                                       ps_eo[:].rearrange("p a b -> p (a b)"))
                    e_sb, o_sb = eo_sb[:, 0, :], eo_sb[:, 1, :]
                    v_proj(2 * pair)
                    v_proj(2 * pair + 1)
                    t1 = rope_pool.tile([128, 512], bf16, tag="t1")
                    t2 = rope_pool.tile([128, 512], bf16, tag="t2")
                    t3 = rope_pool.tile([128, 512], bf16, tag="t3")
                    t4 = rope_pool.tile([128, 512], bf16, tag="t4")
                    nc.vector.tensor_tensor(t1[:], e_sb, cs_c, MUL)
                    nc.vector.tensor_tensor(t2[:], o_sb, sn_c, MUL)
                    nc.vector.tensor_tensor(t3[:], e_sb, sn_c, MUL)
                    nc.vector.tensor_tensor(t4[:], o_sb, cs_c, MUL)
                    # rotate + scatter straight into the per-head tiles
                    dstT = qT if pair == 0 else kT
                    for h in range(4):
                        h2, hh = h // 2, h % 2
                        r0 = hh * 64
                        hs = slice(h * 32, (h + 1) * 32)
                        nc.vector.tensor_tensor(dstT[h2][r0:r0 + 32, sl], t1[hs, :], t2[hs, :], SUB)
                        nc.vector.tensor_tensor(dstT[h2][r0 + 32:r0 + 64, sl], t3[hs, :], t4[hs, :], ADD)

            def era2_attn(qc, hp):
                # both heads of the pair interleaved: their score matmuls use
                # disjoint PE row ranges (0-63 / 64-127) back-to-back, giving
                # the array a chance to overlap the two stationary tiles
                nkt_q = (qc + 1) * 4
                ps_o2 = [ps_o_pool.tile([65, 512], f32, tag="o", name=f"pso{hh}")
                         for hh in range(2)]
                for ki2 in range(nkt_q // 2):
                    ps_s2 = [ps_s_pool.tile([128, 2, 512], f32, tag="s", name=f"pss{hh}")
                             for hh in range(2)]
                    for half in range(2):
                        ki = 2 * ki2 + half
                        diag = ki // 4 == qc
                        soff = max(0, ki * 128 - qc * 512) if diag else 0
                        for hh in range(2):
                            r0 = hh * 64
                            nc.tensor.matmul(
                                ps_s2[hh][:, half, soff:512],
                                kT[hp][r0:r0 + 64, ki * 128:(ki + 1) * 128],
                                qT[hp][r0:r0 + 64, qc * 512 + soff:(qc + 1) * 512],
                                start=True, stop=not diag, skip_group_check=True)
                        if diag:
                            # accumulate -240 above the diagonal so the exp
                            # below masks causally with no extra op
                            for hh in range(2):
                                nc.tensor.matmul(
                                    ps_s2[hh][:, half, soff:soff + 128],
                                    id_sb[:], tri_sb[:],
                                    start=False, stop=True, skip_group_check=True)
                    pts = []
                    for hh in range(2):
                        ps_s = ps_s2[hh]
                        pt = pt_pool.tile([128, 2, 512], bf16, tag="pt", name=f"pt{hh}")
                        pts.append(pt)
                        if (2 * ki2) // 4 != qc and (2 * ki2 + 1) // 4 != qc:
                            nc.scalar.activation(
                                pt[:].rearrange("p a b -> p (a b)"),
                                ps_s[:].rearrange("p a b -> p (a b)"), Exp, scale=0.125)
                        else:
                            for half in range(2):
                                ki = 2 * ki2 + half
                                off = ki * 128 - qc * 512
                                if ki // 4 == qc:   # diagonal tile
                                    nc.scalar.activation(pt[:, half, off:512],
                                                         ps_s[:, half, off:512], Exp, scale=0.125)
                                else:
                                    nc.scalar.activation(pt[:, half, :],
                                                         ps_s[:, half, :], Exp, scale=0.125)
                    for half in range(2):
                        ki = 2 * ki2 + half
                        soff = max(0, ki * 128 - qc * 512) if (ki // 4 == qc and ki != 0) else 0
                        for hh in range(2):
                            h = hp * 2 + hh
                            nc.tensor.matmul(ps_o2[hh][:, soff:512],
                                             v_sb[:, ki, h * 65:(h + 1) * 65],
                                             pts[hh][:, half, soff:512],
                                             start=(ki == 0), stop=(ki == nkt_q - 1),
                                             skip_group_check=True)
                for hh in range(2):
                    r0 = hh * 64
                    # evict ps_o to SBUF immediately (frees the PSUM bank),
                    # then normalize off-critical-path
                    o_cp = nrm_pool.tile([65, 512], f32, tag="ocp")
                    nc.vector.tensor_copy(o_cp[:], ps_o2[hh][:])
                    rrow = nrm_pool.tile([1, 512], f32, tag="rrow")
                    nc.vector.reciprocal(rrow[:], o_cp[64:65, :])
                    bsum = nrm_pool.tile([64, 512], f32, tag="bsum")
                    nc.sync.dma_start(bsum[:], rrow[0:1, None, :].to_broadcast([1, 64, 512]))
                    nc.gpsimd.tensor_tensor(
                        a_sb[hp][r0:r0 + 64, qc * 512:(qc + 1) * 512],
                        o_cp[0:64, :], bsum[:], MUL)

            def out_tile(qc, tl):
                ti = qc * 4 + tl
                psy = ps_s_pool.tile([128, 2, 512], f32, tag="s")
                for ncol in range(2):
                    for kc2 in range(2):
                        nc.tensor.matmul(psy[:, ncol, :], a_sb[kc2][:, ti * 128:(ti + 1) * 128],
                                         wo_sb[:, kc2, ncol * 512:(ncol + 1) * 512],
                                         start=(kc2 == 0), stop=(kc2 == 1),
                                         skip_group_check=True)
                yt = y_pool.tile([128, C], bf16, tag="yt")
                nc.vector.tensor_copy(yt[:], psy[:].rearrange("p a b -> p (a b)"))
                nc.sync.dma_start(y_d[ti * 128:(ti + 1) * 128, :], yt[:])

            for qtr in range(NQTR):
                era1(qtr)
                if 2 in phases:
                    # two out-projection tiles of the previous chunk after
                    # each head pair: the yt/psum round-trip hides behind a
                    # pair's worth of score matmuls
                    for hp in range(2):
                        era2_attn(qtr, hp)
                        if qtr > 0:
                            out_tile(qtr - 1, 2 * hp)
                            out_tile(qtr - 1, 2 * hp + 1)
            if 2 in phases:
                for tl in range(4):
                    out_tile(NQTR - 1, tl)

          for u in range(unroll):
              iter_body(u)

    if split_waits:
        _split_multi_waits(nc)
    return nc


# ---------------------------------------------------------------------------
# Host-side sharding / gather
# ---------------------------------------------------------------------------

def _rope_tables():
    inv_freq = (1.0 / (ROPE_BASE ** (np.arange(0, D, 2, dtype=np.float32) / D))).astype(np.float32)
    ang = (np.arange(T, dtype=np.float32)[:, None] * inv_freq[None, :]).astype(np.float32)  # (T, 32)
    cos, sin = np.cos(ang), np.sin(ang)
    idx = np.arange(128) % 32
    return np.ascontiguousarray(cos[:, idx].T), np.ascontiguousarray(sin[:, idx].T)  # (128, T)


def _perm_cols(g):
    """w_qkv column order for core group g: [QE|QO|KE|KO|V]."""
    cols = []
    for base, par in ((0, 0), (0, 1), (C, 0), (C, 1)):      # QE, QO, KE, KO
        for hl in range(4):
            hg = g * 4 + hl
            for i in range(32):
                cols.append(base + hg * 64 + 2 * i + par)
    for hl in range(4):
        hg = g * 4 + hl
        for d_ in range(64):
            cols.append(2 * C + hg * 64 + d_)
    return np.asarray(cols)


def make_in_maps(x, w_qkv, w_out):
    import ml_dtypes
    bf16 = ml_dtypes.bfloat16
    x = np.asarray(x, np.float32)
    w_qkv = np.asarray(w_qkv, np.float32)
    w_out = np.asarray(w_out, np.float32)
    cs, sn = _rope_tables()
    # additive causal mask for S^T[k, q] diagonal blocks: 0 iff q >= k
    tri = np.where(np.tril(np.ones((128, 128), np.float32)).T > 0, 0.0, -240.0
                   ).astype(np.float32)
    ident = np.eye(128, dtype=np.float32)
    in_maps = []
    for c in range(NCORES):
        b, g = c // 4, c % 4
        in_maps.append({
            "x": np.ascontiguousarray(x[b]).astype(bf16),
            "w": np.ascontiguousarray(w_qkv[:, _perm_cols(g)]).astype(bf16),
            "wo": np.ascontiguousarray(w_out[g * 256:(g + 1) * 256, :]).astype(bf16),
            "cs": cs.astype(bf16), "sn": sn.astype(bf16),
            "tri": tri.astype(bf16), "id": ident.astype(bf16),
        })
    return in_maps


def kernel(x, w_qkv, w_out):
    global _BUILT
    from concourse.bass_utils import run_bass_kernel_spmd

    if _BUILT is None:
        _BUILT = build_nc()
    in_maps = make_in_maps(x, w_qkv, w_out)
    res = run_bass_kernel_spmd(_BUILT, in_maps, core_ids=list(range(NCORES)))
    out = np.zeros((B, T, C), np.float32)
    for c in range(NCORES):
        out[c // 4] += res.results[c]["y"].astype(np.float32)
    return out



# revision 2
# speedup vs baseline: 1.1770x; 1.1770x over previous
"""Multi-head self-attention (B=2, T=2048, C=1024, H=16, RoPE, causal) on 8 trn2 cores.

v8: fused bf16 pipeline, hw-latency-tuned.
  - Sharding: data-parallel over batch (2) x tensor-parallel over head groups (4).
    Core c handles batch c//4, heads (c%4)*4 .. +3; host sums 4 partials/batch.
  - All inputs host-cast to bf16; x is HOST-TRANSPOSED to [C, T] so x^T tiles
    load as plain DMAs (no DMA-XBAR transposes).
  - era1 (projections+RoPE) and era2 (attention) fused per 512-row quarter.
  - era2 is ki-granular: one [128, hh, 512] psum tile per k-tile holds BOTH
    heads of the pair, so the 2-slot psum ring double-buffers across ki and
    scores(ki+1) overlap exp(ki); AV(ki) is emitted after scores(ki+1) so PE
    stays busy while Act computes the exp.
  - v tiles carry 64 ONES-COLUMNS per head ([128, kt, 4, 64v+64ones]): the AV
    matmul lands the softmax row-sum REPLICATED across psum partitions
    64..127 at zero extra PE cost (matmul time scales only with moving
    columns), so normalization is reciprocal+multiply straight out of PSUM on
    DVE — no broadcast DMA, no psum eviction for the sums.
  - Timing loop: unroll=4 bodies per For_i trip amortizes the per-trip tile
    pool drain (~60us/trip on hw).
"""
import sys
import math

sys.path.insert(0, "/opt/trn_rl_repo")

import numpy as np

B, T, C, H, D = 2, 2048, 1024, 16, 64
NCORES = 8
NKC = C // 128         # 8 contraction chunks
NQTR = T // 512        # 4 t-quarters
NKT = T // 128         # 16 k-tiles
ROPE_BASE = 10000.0

_BUILT = None


# ---------------------------------------------------------------------------
# Toolchain workaround: this walrus build accepts at most ONE semaphore wait
# per instruction.  (a) replace Tile's exit drain with a chain of single-wait
# drains; (b) hoist extra waits onto same-engine nops.
# ---------------------------------------------------------------------------

def _apply_tile_patch():
    import bass_rust
    import concourse.tile as tile
    from concourse.vector_clock import ScopedClock

    def _patched_drain_and_barrier(self, tick_clock, wait_clock):
        nc = self.nc
        probe = nc.sync.drain()
        wait_clock.add_sem_waits(probe.ins, ScopedClock({None: tick_clock.global_clock}))
        si = probe.ins.sync_info
        waits = list(si.on_wait) if si is not None else []
        probe.ins.sync_info = None
        name2sem = {s.name: s for s in wait_clock.sems.allocated().values()}
        for w in waits:
            d = nc.sync.drain()
            bass_rust.wait_op(d.ins, name2sem[w.ant_name], w.wait_value, "sem-ge", False)
        nc.all_engine_barrier()
        popped = nc._tile_sem_poison_stack.pop()
        assert popped is self._sem_poison
        nc.clear_and_free_semaphores(list(self.sems.allocated().values()))
        nc.all_engine_barrier()

    tile.TileContext._drain_and_barrier = _patched_drain_and_barrier


def _split_multi_waits(nc):
    import bass_rust
    import concourse.mybir as mybir

    ctr = 0
    for fn in nc.m.functions:
        for blk in fn.blocks:
            il = blk.instructions
            new = []
            changed = False
            for inst in il:
                si = inst.sync_info
                waits = list(si.on_wait) if si is not None else []
                if len(waits) > 1:
                    changed = True
                    for w in waits[:-1]:
                        nop = mybir.InstNoOp(name=f"I-waitsplit-{ctr}", ins=[], outs=[])
                        ctr += 1
                        nop.engine = inst.engine
                        nop.sync_info = bass_rust.SyncInfo(on_wait=[w], on_update=[])
                        new.append(nop)
                    inst.sync_info = bass_rust.SyncInfo(
                        on_wait=[waits[-1]], on_update=list(si.on_update)
                    )
                new.append(inst)
            if changed:
                blk.instructions = new


# ---------------------------------------------------------------------------
# Kernel builder (per-core program; identical on all 8 cores)
# ---------------------------------------------------------------------------

def build_nc(split_waits=True, loop_iters=None, phases=(1, 2), unroll=None):
    if unroll is None:
        if loop_iters:
            unroll = 4 if loop_iters % 4 == 0 else (2 if loop_iters % 2 == 0 else 1)
        else:
            unroll = 1
    if loop_iters:
        assert loop_iters % unroll == 0
        loop_iters = loop_iters // unroll
    _apply_tile_patch()
    import concourse.bass as bass
    import concourse.tile as tile
    import concourse.mybir as mybir
    from contextlib import nullcontext

    dt = mybir.dt
    f32, bf16 = dt.float32, dt.bfloat16
    Exp = mybir.ActivationFunctionType.Exp
    MUL, SUB, ADD = (mybir.AluOpType.mult, mybir.AluOpType.subtract,
                     mybir.AluOpType.add)

    nc = bass.Bass()
    x_d = nc.dram_tensor("x", [C, T], bf16, kind="ExternalInput")  # host-transposed
    w_d = nc.dram_tensor("w", [C, 768], bf16, kind="ExternalInput")
    wo_d = nc.dram_tensor("wo", [256, C], bf16, kind="ExternalInput")
    cs_d = nc.dram_tensor("cs", [128, T], bf16, kind="ExternalInput")
    sn_d = nc.dram_tensor("sn", [128, T], bf16, kind="ExternalInput")
    # tri is the additive causal mask for S^T diagonal blocks (0 on/below
    # diagonal, -240 above); id is a 128x128 identity for the mask matmul
    tri_d = nc.dram_tensor("tri", [128, 128], bf16, kind="ExternalInput")
    id_d = nc.dram_tensor("id", [128, 128], bf16, kind="ExternalInput")
    y_d = nc.dram_tensor("y", [T, C], bf16, kind="ExternalOutput")

    with tile.TileContext(nc) as tc:
      loop_cm = (tc.For_i(0, loop_iters, 1,
                          hint_engines=(mybir.EngineType.PE, mybir.EngineType.Activation,
                                        mybir.EngineType.DVE, mybir.EngineType.SP,
                                        mybir.EngineType.Pool))
                 if loop_iters else nullcontext())
      with loop_cm:
        with (
            tc.tile_pool(name="persist", bufs=1) as persist,
            tc.tile_pool(name="vsb", bufs=2) as vsb_pool,
            tc.tile_pool(name="qkT", bufs=2) as qkT_pool,
            tc.tile_pool(name="asb", bufs=2) as asb_pool,
            tc.tile_pool(name="w", bufs=1) as w_pool,
            tc.tile_pool(name="xT", bufs=2) as xT_pool,
            tc.tile_pool(name="rope", bufs=2) as rope_pool,
            tc.tile_pool(name="pt", bufs=3) as pt_pool,
            tc.tile_pool(name="nrm", bufs=2) as nrm_pool,
            tc.tile_pool(name="yout", bufs=2) as y_pool,
            tc.tile_pool(name="pconst", bufs=1) as pconst,
            tc.tile_pool(name="ps_a", bufs=1, space="PSUM") as ps_a_pool,
                        tc.tile_pool(name="ps_s", bufs=2, space="PSUM") as ps_s_pool,
            tc.tile_pool(name="ps_o", bufs=2, space="PSUM") as ps_o_pool,
        ):
          def iter_body(u):
            # per-head-contiguous rotated q^T/k^T: tile [128, T] = 2 heads,
            # rows [h_ev(32); h_od(32); h'_ev(32); h'_od(32)].  bufs=2 pools
            # alternate buffers between the two unrolled bodies so iteration
            # i+1's era1 overlaps iteration i's era2.
            qT = [qkT_pool.tile([128, T], bf16, tag=f"qT{i}", name=f"qT{i}_{u}") for i in range(2)]
            kT = [qkT_pool.tile([128, T], bf16, tag=f"kT{i}", name=f"kT{i}_{u}") for i in range(2)]
            # v in (t, d) layout + 64 ones-columns per head slot
            # ([128, kt, 4, 64+64]): the AV matmul then lands the softmax
            # row-sum REPLICATED across psum partitions 64..127, so the
            # normalize needs no partition broadcast at all
            v_sb = vsb_pool.tile([128, NKT, 4, 128], bf16, tag="v")
            # PE-consumed constants double-buffered: their per-body reload
            # must not WAR-block on the previous body's tail reads
            wo_sb = pconst.tile([128, 2, C], bf16, tag="wo")
            tri_sb = pconst.tile([128, 128], bf16, tag="tri")
            id_sb = pconst.tile([128, 128], bf16, tag="id")
            cs_sb = persist.tile([128, T], bf16, tag="cs")
            sn_sb = persist.tile([128, T], bf16, tag="sn")
            w_sb = w_pool.tile([128, NKC, 768], bf16, tag="w")
            a_sb = [asb_pool.tile([128, T], bf16, tag=f"a{i}", name=f"a{i}_{u}") for i in range(2)]

            v4 = v_sb
            nc.gpsimd.memset(v4[:, :, :, 64:128], 1.0)

            def era1(qtr):
                xT_q = xT_pool.tile([128, NKC, 512], bf16, tag="xTq")
                for kc in range(NKC):
                    if qtr == 0 and kc % 2 == 0:
                        # interleave w chunks with the x transposes so the
                        # first projection matmuls can start early
                        nc.sync.dma_start(
                            w_sb[:, kc:kc + 2, :],
                            w_d[kc * 128:(kc + 2) * 128, :].rearrange("(kc p) f -> p kc f", p=128))
                    nc.sync.dma_start(
                        xT_q[:, kc, :],
                        x_d[kc * 128:(kc + 1) * 128, qtr * 512:(qtr + 1) * 512])
                if qtr == 0:
                    nc.sync.dma_start(cs_sb[:], cs_d[:])
                    nc.sync.dma_start(sn_sb[:], sn_d[:])
                    nc.sync.dma_start(tri_sb[:], tri_d[:])
                    nc.sync.dma_start(id_sb[:], id_d[:])
                    nc.sync.dma_start(wo_sb[:], wo_d[:].rearrange("(kc p) c -> p kc c", p=128))

                # ---- QK projection + RoPE (pairs: (QE,QO) then (KE,KO)),
                # V-projection tiles interleaved to cover eviction latency
                cs_c = cs_sb[:, qtr * 512:(qtr + 1) * 512]
                sn_c = sn_sb[:, qtr * 512:(qtr + 1) * 512]
                sl = slice(qtr * 512, (qtr + 1) * 512)

                def v_proj(tl):
                    psv = ps_s_pool.tile([128, 2, 512], f32, tag="s")
                    for kc in range(NKC):
                        nc.tensor.matmul(psv[:, 0, 0:256], xT_q[:, kc, tl * 128:(tl + 1) * 128],
                                         w_sb[:, kc, 512:768], start=(kc == 0), stop=(kc == NKC - 1),
                                         skip_group_check=True)
                    kt = qtr * 4 + tl
                    # Act-engine copy: 'copy' shares the exp activation table,
                    # so no table reloads alternate with era2's exps
                    nc.scalar.copy(v4[:, kt, :, 0:64],
                                   psv[:, 0, 0:256].rearrange("p (h d) -> p h d", h=4))

                for pair in range(2):          # 0: Q, 1: K
                    m_e, m_o = 2 * pair, 2 * pair + 1
                    ps_eo = ps_a_pool.tile([128, 2, 512], f32, tag="proj")
                    for kc in range(NKC):
                        nc.tensor.matmul(ps_eo[:, 0, :], w_sb[:, kc, m_e * 128:(m_e + 1) * 128],
                                         xT_q[:, kc, :], start=(kc == 0), stop=(kc == NKC - 1),
                                         skip_group_check=True)
                        nc.tensor.matmul(ps_eo[:, 1, :], w_sb[:, kc, m_o * 128:(m_o + 1) * 128],
                                         xT_q[:, kc, :], start=(kc == 0), stop=(kc == NKC - 1),
                                         skip_group_check=True)
                    # PSUM eviction in one shot; rope math on DVE (2-byte 2x
                    # mode, ~3.4x faster than GPSIMD's software TT).  At the
                    # body boundary (qtr 0) Act is still draining the previous
                    # body's exp backlog, so evict on DVE there instead.
                    eo_sb = rope_pool.tile([128, 2, 512], bf16, tag="eo")
                    if qtr == 0:
                        nc.vector.tensor_copy(eo_sb[:].rearrange("p a b -> p (a b)"),
                                              ps_eo[:].rearrange("p a b -> p (a b)"))
                    else:
                        nc.scalar.copy(eo_sb[:].rearrange("p a b -> p (a b)"),
                                       ps_eo[:].rearrange("p a b -> p (a b)"))
                    e_sb, o_sb = eo_sb[:, 0, :], eo_sb[:, 1, :]
                    v_proj(2 * pair)
                    v_proj(2 * pair + 1)
                    t1 = rope_pool.tile([128, 512], bf16, tag="t1")
                    t2 = rope_pool.tile([128, 512], bf16, tag="t2")
                    t3 = rope_pool.tile([128, 512], bf16, tag="t3")
                    t4 = rope_pool.tile([128, 512], bf16, tag="t4")
                    nc.vector.tensor_tensor(t1[:], e_sb, cs_c, MUL)
                    nc.vector.tensor_tensor(t2[:], o_sb, sn_c, MUL)
                    nc.vector.tensor_tensor(t3[:], e_sb, sn_c, MUL)
                    nc.vector.tensor_tensor(t4[:], o_sb, cs_c, MUL)
                    # rotate + scatter straight into the per-head tiles
                    dstT = qT if pair == 0 else kT
                    for h in range(4):
                        h2, hh = h // 2, h % 2
                        r0 = hh * 64
                        hs = slice(h * 32, (h + 1) * 32)
                        nc.vector.tensor_tensor(dstT[h2][r0:r0 + 32, sl], t1[hs, :], t2[hs, :], SUB)
                        nc.vector.tensor_tensor(dstT[h2][r0 + 32:r0 + 64, sl], t3[hs, :], t4[hs, :], ADD)

            def era2_attn(qc, hp):
                # ki-granular pipeline: one psum tile per k-tile holds BOTH
                # heads ([*, hh, cols]) so the bufs=2 "s" ring double-buffers
                # across ki steps and exp(ki) pre-satisfies AV(ki)'s wait
                nkt_q = (qc + 1) * 4
                ps_o2 = [ps_o_pool.tile([128, 512], f32, tag="o", name=f"pso{hh}")
                         for hh in range(2)]
                def emit_av(ki, pt, soff):
                    for hh in range(2):
                        h = hp * 2 + hh
                        nc.tensor.matmul(ps_o2[hh][:, soff:512],
                                         v_sb[:, ki, h, :],
                                         pt[:, hh, soff:512],
                                         start=(ki == 0), stop=(ki == nkt_q - 1),
                                         skip_group_check=True)

                pend = None   # (ki, pt, soff) whose AV is deferred one step
                for ki in range(nkt_q):
                    diag = ki // 4 == qc
                    soff = max(0, ki * 128 - qc * 512) if diag else 0
                    ps_s = ps_s_pool.tile([128, 2, 512], f32, tag="s")
                    for hh in range(2):
                        r0 = hh * 64
                        nc.tensor.matmul(
                            ps_s[:, hh, soff:512],
                            kT[hp][r0:r0 + 64, ki * 128:(ki + 1) * 128],
                            qT[hp][r0:r0 + 64, qc * 512 + soff:(qc + 1) * 512],
                            start=True, stop=not diag, skip_group_check=True)
                    if diag:
                        for hh in range(2):
                            nc.tensor.matmul(
                                ps_s[:, hh, soff:soff + 128],
                                id_sb[:], tri_sb[:],
                                start=False, stop=True, skip_group_check=True)
                    pt = pt_pool.tile([128, 2, 512], bf16, tag="pt")
                    if diag and soff:
                        for hh in range(2):
                            nc.scalar.activation(pt[:, hh, soff:512],
                                                 ps_s[:, hh, soff:512], Exp, scale=0.125)
                    else:
                        nc.scalar.activation(
                            pt[:].rearrange("p a b -> p (a b)"),
                            ps_s[:].rearrange("p a b -> p (a b)"), Exp, scale=0.125)
                    # defer AV(ki) until after scores(ki+1): PE stays busy
                    # while Act computes exp(ki)
                    if pend is not None:
                        emit_av(*pend)
                    pend = (ki, pt, soff)
                if pend is not None:
                    emit_av(*pend)
                for hh in range(2):
                    r0 = hh * 64
                    # sums arrive replicated in psum partitions 64..127, so
                    # normalize is just recip + multiply on DVE — no
                    # broadcast DMA, no eviction
                    rrec = nrm_pool.tile([64, 512], f32, tag="rrec")
                    nc.vector.reciprocal(rrec[:], ps_o2[hh][64:128, :])
                    nc.vector.tensor_tensor(
                        a_sb[hp][r0:r0 + 64, qc * 512:(qc + 1) * 512],
                        ps_o2[hh][0:64, :], rrec[:], MUL)

            def out_tile(qc, tl):
                ti = qc * 4 + tl
                psy = ps_s_pool.tile([128, 2, 512], f32, tag="s")
                for ncol in range(2):
                    for kc2 in range(2):
                        nc.tensor.matmul(psy[:, ncol, :], a_sb[kc2][:, ti * 128:(ti + 1) * 128],
                                         wo_sb[:, kc2, ncol * 512:(ncol + 1) * 512],
                                         start=(kc2 == 0), stop=(kc2 == 1),
                                         skip_group_check=True)
                yt = y_pool.tile([128, C], bf16, tag="yt")
                nc.vector.tensor_copy(yt[:], psy[:].rearrange("p a b -> p (a b)"))
                nc.sync.dma_start(y_d[ti * 128:(ti + 1) * 128, :], yt[:])

            for qtr in range(NQTR):
                era1(qtr)
                if 2 in phases:
                    # two out-projection tiles of the previous chunk after
                    # each head pair: the yt/psum round-trip hides behind a
                    # pair's worth of score matmuls
                    for hp in range(2):
                        era2_attn(qtr, hp)
                        if qtr > 0:
                            out_tile(qtr - 1, 2 * hp)
                            out_tile(qtr - 1, 2 * hp + 1)
            if 2 in phases:
                for tl in range(4):
                    out_tile(NQTR - 1, tl)

          for u in range(unroll):
              iter_body(u)

    if split_waits:
        _split_multi_waits(nc)
    return nc


# ---------------------------------------------------------------------------
# Host-side sharding / gather
# ---------------------------------------------------------------------------

def _rope_tables():
    inv_freq = (1.0 / (ROPE_BASE ** (np.arange(0, D, 2, dtype=np.float32) / D))).astype(np.float32)
    ang = (np.arange(T, dtype=np.float32)[:, None] * inv_freq[None, :]).astype(np.float32)  # (T, 32)
    cos, sin = np.cos(ang), np.sin(ang)
    idx = np.arange(128) % 32
    return np.ascontiguousarray(cos[:, idx].T), np.ascontiguousarray(sin[:, idx].T)  # (128, T)


def _perm_cols(g):
    """w_qkv column order for core group g: [QE|QO|KE|KO|V]."""
    cols = []
    for base, par in ((0, 0), (0, 1), (C, 0), (C, 1)):      # QE, QO, KE, KO
        for hl in range(4):
            hg = g * 4 + hl
            for i in range(32):
                cols.append(base + hg * 64 + 2 * i + par)
    for hl in range(4):
        hg = g * 4 + hl
        for d_ in range(64):
            cols.append(2 * C + hg * 64 + d_)
    return np.asarray(cols)


def make_in_maps(x, w_qkv, w_out):
    import ml_dtypes
    bf16 = ml_dtypes.bfloat16
    x = np.asarray(x, np.float32)
    w_qkv = np.asarray(w_qkv, np.float32)
    w_out = np.asarray(w_out, np.float32)
    cs, sn = _rope_tables()
    # additive causal mask for S^T[k, q] diagonal blocks: 0 iff q >= k
    tri = np.where(np.tril(np.ones((128, 128), np.float32)).T > 0, 0.0, -240.0
                   ).astype(np.float32)
    ident = np.eye(128, dtype=np.float32)
    in_maps = []
    for c in range(NCORES):
        b, g = c // 4, c % 4
        in_maps.append({
            "x": np.ascontiguousarray(x[b].T).astype(bf16),
            "w": np.ascontiguousarray(w_qkv[:, _perm_cols(g)]).astype(bf16),
            "wo": np.ascontiguousarray(w_out[g * 256:(g + 1) * 256, :]).astype(bf16),
            "cs": cs.astype(bf16), "sn": sn.astype(bf16),
            "tri": tri.astype(bf16), "id": ident.astype(bf16),
        })
    return in_maps


def kernel(x, w_qkv, w_out):
    global _BUILT
    from concourse.bass_utils import run_bass_kernel_spmd

    if _BUILT is None:
        _BUILT = build_nc()
    in_maps = make_in_maps(x, w_qkv, w_out)
    res = run_bass_kernel_spmd(_BUILT, in_maps, core_ids=list(range(NCORES)))
    out = np.zeros((B, T, C), np.float32)
    for c in range(NCORES):
        out[c // 4] += res.results[c]["y"].astype(np.float32)
    return out



# revision 3
# speedup vs baseline: 1.2108x; 1.0287x over previous
"""Multi-head self-attention (B=2, T=2048, C=1024, H=16, RoPE, causal) on 8 trn2 cores.

v8: fused bf16 pipeline, hw-latency-tuned.
  - Sharding: data-parallel over batch (2) x tensor-parallel over head groups (4).
    Core c handles batch c//4, heads (c%4)*4 .. +3; host sums 4 partials/batch.
  - All inputs host-cast to bf16; x is HOST-TRANSPOSED to [C, T] so x^T tiles
    load as plain DMAs (no DMA-XBAR transposes).
  - era1 (projections+RoPE) and era2 (attention) fused per 512-row quarter.
  - era2 is ki-granular: one [128, hh, 512] psum tile per k-tile holds BOTH
    heads of the pair, so the 2-slot psum ring double-buffers across ki and
    scores(ki+1) overlap exp(ki); AV(ki) is emitted after scores(ki+1) so PE
    stays busy while Act computes the exp.
  - v tiles carry 64 ONES-COLUMNS per head ([128, kt, 4, 64v+64ones]): the AV
    matmul lands the softmax row-sum REPLICATED across psum partitions
    64..127 at zero extra PE cost (matmul time scales only with moving
    columns), so normalization is reciprocal+multiply straight out of PSUM on
    DVE — no broadcast DMA, no psum eviction for the sums.
  - Timing loop: unroll=4 bodies per For_i trip amortizes the per-trip tile
    pool drain (~60us/trip on hw).
"""
import sys
import math

sys.path.insert(0, "/opt/trn_rl_repo")

import numpy as np

B, T, C, H, D = 2, 2048, 1024, 16, 64
NCORES = 8
NKC = C // 128         # 8 contraction chunks
NQTR = T // 512        # 4 t-quarters
NKT = T // 128         # 16 k-tiles
ROPE_BASE = 10000.0

_BUILT = None


# ---------------------------------------------------------------------------
# Toolchain workaround: this walrus build accepts at most ONE semaphore wait
# per instruction.  (a) replace Tile's exit drain with a chain of single-wait
# drains; (b) hoist extra waits onto same-engine nops.
# ---------------------------------------------------------------------------

def _apply_tile_patch():
    import bass_rust
    import concourse.tile as tile
    from concourse.vector_clock import ScopedClock

    def _patched_drain_and_barrier(self, tick_clock, wait_clock):
        nc = self.nc
        probe = nc.sync.drain()
        wait_clock.add_sem_waits(probe.ins, ScopedClock({None: tick_clock.global_clock}))
        si = probe.ins.sync_info
        waits = list(si.on_wait) if si is not None else []
        probe.ins.sync_info = None
        name2sem = {s.name: s for s in wait_clock.sems.allocated().values()}
        for w in waits:
            d = nc.sync.drain()
            bass_rust.wait_op(d.ins, name2sem[w.ant_name], w.wait_value, "sem-ge", False)
        nc.all_engine_barrier()
        popped = nc._tile_sem_poison_stack.pop()
        assert popped is self._sem_poison
        nc.clear_and_free_semaphores(list(self.sems.allocated().values()))
        nc.all_engine_barrier()

    tile.TileContext._drain_and_barrier = _patched_drain_and_barrier


def _split_multi_waits(nc):
    import bass_rust
    import concourse.mybir as mybir

    ctr = 0
    for fn in nc.m.functions:
        for blk in fn.blocks:
            il = blk.instructions
            new = []
            changed = False
            for inst in il:
                si = inst.sync_info
                waits = list(si.on_wait) if si is not None else []
                if len(waits) > 1:
                    changed = True
                    for w in waits[:-1]:
                        nop = mybir.InstNoOp(name=f"I-waitsplit-{ctr}", ins=[], outs=[])
                        ctr += 1
                        nop.engine = inst.engine
                        nop.sync_info = bass_rust.SyncInfo(on_wait=[w], on_update=[])
                        new.append(nop)
                    inst.sync_info = bass_rust.SyncInfo(
                        on_wait=[waits[-1]], on_update=list(si.on_update)
                    )
                new.append(inst)
            if changed:
                blk.instructions = new


# ---------------------------------------------------------------------------
# Kernel builder (per-core program; identical on all 8 cores)
# ---------------------------------------------------------------------------

def build_nc(split_waits=True, loop_iters=None, phases=(1, 2), unroll=None):
    if unroll is None:
        # deepest unroll the trip count supports: the per-trip pool drain
        # (~60us on hw) amortizes across unrolled bodies
        if loop_iters:
            unroll = next(u for u in (8, 4, 2, 1) if loop_iters % u == 0)
        else:
            unroll = 1
    if loop_iters:
        assert loop_iters % unroll == 0
        loop_iters = loop_iters // unroll
    _apply_tile_patch()
    import concourse.bass as bass
    import concourse.tile as tile
    import concourse.mybir as mybir
    from contextlib import nullcontext

    dt = mybir.dt
    f32, bf16 = dt.float32, dt.bfloat16
    Exp = mybir.ActivationFunctionType.Exp
    MUL, SUB, ADD = (mybir.AluOpType.mult, mybir.AluOpType.subtract,
                     mybir.AluOpType.add)

    nc = bass.Bass()
    x_d = nc.dram_tensor("x", [C, T], bf16, kind="ExternalInput")  # host-transposed
    w_d = nc.dram_tensor("w", [C, 768], bf16, kind="ExternalInput")
    wo_d = nc.dram_tensor("wo", [256, C], bf16, kind="ExternalInput")
    cs_d = nc.dram_tensor("cs", [128, T], bf16, kind="ExternalInput")
    sn_d = nc.dram_tensor("sn", [128, T], bf16, kind="ExternalInput")
    # tri is the additive causal mask for S^T diagonal blocks (0 on/below
    # diagonal, -240 above); id is a 128x128 identity for the mask matmul
    tri_d = nc.dram_tensor("tri", [128, 128], bf16, kind="ExternalInput")
    id_d = nc.dram_tensor("id", [128, 128], bf16, kind="ExternalInput")
    y_d = nc.dram_tensor("y", [T, C], bf16, kind="ExternalOutput")

    with tile.TileContext(nc) as tc:
      loop_cm = (tc.For_i(0, loop_iters, 1,
                          hint_engines=(mybir.EngineType.PE, mybir.EngineType.Activation,
                                        mybir.EngineType.DVE, mybir.EngineType.SP,
                                        mybir.EngineType.Pool))
                 if loop_iters else nullcontext())
      with loop_cm:
        with (
            tc.tile_pool(name="persist", bufs=1) as persist,
            tc.tile_pool(name="vsb", bufs=2) as vsb_pool,
            tc.tile_pool(name="qkT", bufs=2) as qkT_pool,
            tc.tile_pool(name="asb", bufs=2) as asb_pool,
            tc.tile_pool(name="w", bufs=1) as w_pool,
            tc.tile_pool(name="xT", bufs=2) as xT_pool,
            tc.tile_pool(name="rope", bufs=2) as rope_pool,
            tc.tile_pool(name="pt", bufs=3) as pt_pool,
            tc.tile_pool(name="nrm", bufs=2) as nrm_pool,
            tc.tile_pool(name="yout", bufs=2) as y_pool,
            tc.tile_pool(name="pconst", bufs=1) as pconst,
            tc.tile_pool(name="ps_a", bufs=1, space="PSUM") as ps_a_pool,
                        tc.tile_pool(name="ps_s", bufs=2, space="PSUM") as ps_s_pool,
            tc.tile_pool(name="ps_o", bufs=2, space="PSUM") as ps_o_pool,
        ):
          def iter_body(u):
            # per-head-contiguous rotated q^T/k^T: tile [128, T] = 2 heads,
            # rows [h_ev(32); h_od(32); h'_ev(32); h'_od(32)].  bufs=2 pools
            # alternate buffers between the two unrolled bodies so iteration
            # i+1's era1 overlaps iteration i's era2.
            qT = [qkT_pool.tile([128, T], bf16, tag=f"qT{i}", name=f"qT{i}_{u}") for i in range(2)]
            kT = [qkT_pool.tile([128, T], bf16, tag=f"kT{i}", name=f"kT{i}_{u}") for i in range(2)]
            # v in (t, d) layout + 64 ones-columns per head slot
            # ([128, kt, 4, 64+64]): the AV matmul then lands the softmax
            # row-sum REPLICATED across psum partitions 64..127, so the
            # normalize needs no partition broadcast at all
            v_sb = vsb_pool.tile([128, NKT, 4, 128], bf16, tag="v")
            # PE-consumed constants double-buffered: their per-body reload
            # must not WAR-block on the previous body's tail reads
            wo_sb = pconst.tile([128, 2, C], bf16, tag="wo")
            tri_sb = pconst.tile([128, 128], bf16, tag="tri")
            id_sb = pconst.tile([128, 128], bf16, tag="id")
            cs_sb = persist.tile([128, T], bf16, tag="cs")
            sn_sb = persist.tile([128, T], bf16, tag="sn")
            w_sb = w_pool.tile([128, NKC, 768], bf16, tag="w")
            a_sb = [asb_pool.tile([128, T], bf16, tag=f"a{i}", name=f"a{i}_{u}") for i in range(2)]

            v4 = v_sb
            nc.gpsimd.memset(v4[:, :, :, 64:128], 1.0)

            def era1(qtr):
                xT_q = xT_pool.tile([128, NKC, 512], bf16, tag="xTq")
                for kc in range(NKC):
                    if qtr == 0 and kc % 2 == 0:
                        # interleave w chunks with the x transposes so the
                        # first projection matmuls can start early
                        nc.sync.dma_start(
                            w_sb[:, kc:kc + 2, :],
                            w_d[kc * 128:(kc + 2) * 128, :].rearrange("(kc p) f -> p kc f", p=128))
                    nc.sync.dma_start(
                        xT_q[:, kc, :],
                        x_d[kc * 128:(kc + 1) * 128, qtr * 512:(qtr + 1) * 512])
                if qtr == 0:
                    nc.sync.dma_start(cs_sb[:], cs_d[:])
                    nc.sync.dma_start(sn_sb[:], sn_d[:])
                    nc.sync.dma_start(tri_sb[:], tri_d[:])
                    nc.sync.dma_start(id_sb[:], id_d[:])
                    nc.sync.dma_start(wo_sb[:], wo_d[:].rearrange("(kc p) c -> p kc c", p=128))

                # ---- QK projection + RoPE (pairs: (QE,QO) then (KE,KO)),
                # V-projection tiles interleaved to cover eviction latency
                cs_c = cs_sb[:, qtr * 512:(qtr + 1) * 512]
                sn_c = sn_sb[:, qtr * 512:(qtr + 1) * 512]
                sl = slice(qtr * 512, (qtr + 1) * 512)

                def v_proj(tl):
                    psv = ps_s_pool.tile([128, 2, 512], f32, tag="s")
                    for kc in range(NKC):
                        nc.tensor.matmul(psv[:, 0, 0:256], xT_q[:, kc, tl * 128:(tl + 1) * 128],
                                         w_sb[:, kc, 512:768], start=(kc == 0), stop=(kc == NKC - 1),
                                         skip_group_check=True)
                    kt = qtr * 4 + tl
                    # Act-engine copy: 'copy' shares the exp activation table,
                    # so no table reloads alternate with era2's exps
                    nc.scalar.copy(v4[:, kt, :, 0:64],
                                   psv[:, 0, 0:256].rearrange("p (h d) -> p h d", h=4))

                for pair in range(2):          # 0: Q, 1: K
                    m_e, m_o = 2 * pair, 2 * pair + 1
                    ps_eo = ps_a_pool.tile([128, 2, 512], f32, tag="proj")
                    for kc in range(NKC):
                        nc.tensor.matmul(ps_eo[:, 0, :], w_sb[:, kc, m_e * 128:(m_e + 1) * 128],
                                         xT_q[:, kc, :], start=(kc == 0), stop=(kc == NKC - 1),
                                         skip_group_check=True)
                        nc.tensor.matmul(ps_eo[:, 1, :], w_sb[:, kc, m_o * 128:(m_o + 1) * 128],
                                         xT_q[:, kc, :], start=(kc == 0), stop=(kc == NKC - 1),
                                         skip_group_check=True)
                    # PSUM eviction in one shot; rope math on DVE (2-byte 2x
                    # mode, ~3.4x faster than GPSIMD's software TT).  At the
                    # body boundary (qtr 0) Act is still draining the previous
                    # body's exp backlog, so evict on DVE there instead.
                    eo_sb = rope_pool.tile([128, 2, 512], bf16, tag="eo")
                    if qtr == 0:
                        nc.vector.tensor_copy(eo_sb[:].rearrange("p a b -> p (a b)"),
                                              ps_eo[:].rearrange("p a b -> p (a b)"))
                    else:
                        nc.scalar.copy(eo_sb[:].rearrange("p a b -> p (a b)"),
                                       ps_eo[:].rearrange("p a b -> p (a b)"))
                    e_sb, o_sb = eo_sb[:, 0, :], eo_sb[:, 1, :]
                    v_proj(2 * pair)
                    v_proj(2 * pair + 1)
                    t1 = rope_pool.tile([128, 512], bf16, tag="t1")
                    t2 = rope_pool.tile([128, 512], bf16, tag="t2")
                    t3 = rope_pool.tile([128, 512], bf16, tag="t3")
                    t4 = rope_pool.tile([128, 512], bf16, tag="t4")
                    nc.vector.tensor_tensor(t1[:], e_sb, cs_c, MUL)
                    nc.vector.tensor_tensor(t2[:], o_sb, sn_c, MUL)
                    nc.vector.tensor_tensor(t3[:], e_sb, sn_c, MUL)
                    nc.vector.tensor_tensor(t4[:], o_sb, cs_c, MUL)
                    # rotate + scatter straight into the per-head tiles
                    dstT = qT if pair == 0 else kT
                    for h in range(4):
                        h2, hh = h // 2, h % 2
                        r0 = hh * 64
                        hs = slice(h * 32, (h + 1) * 32)
                        nc.vector.tensor_tensor(dstT[h2][r0:r0 + 32, sl], t1[hs, :], t2[hs, :], SUB)
                        nc.vector.tensor_tensor(dstT[h2][r0 + 32:r0 + 64, sl], t3[hs, :], t4[hs, :], ADD)

            def era2_attn(qc, hp):
                # ki-granular pipeline: one psum tile per k-tile holds BOTH
                # heads ([*, hh, cols]) so the bufs=2 "s" ring double-buffers
                # across ki steps and exp(ki) pre-satisfies AV(ki)'s wait
                nkt_q = (qc + 1) * 4
                ps_o2 = [ps_o_pool.tile([128, 512], f32, tag="o", name=f"pso{hh}")
                         for hh in range(2)]
                def emit_av(ki, pt, soff):
                    for hh in range(2):
                        h = hp * 2 + hh
                        nc.tensor.matmul(ps_o2[hh][:, soff:512],
                                         v_sb[:, ki, h, :],
                                         pt[:, hh, soff:512],
                                         start=(ki == 0), stop=(ki == nkt_q - 1),
                                         skip_group_check=True)

                pend = None   # (ki, pt, soff) whose AV is deferred one step
                for ki in range(nkt_q):
                    diag = ki // 4 == qc
                    soff = max(0, ki * 128 - qc * 512) if diag else 0
                    ps_s = ps_s_pool.tile([128, 2, 512], f32, tag="s")
                    for hh in range(2):
                        r0 = hh * 64
                        nc.tensor.matmul(
                            ps_s[:, hh, soff:512],
                            kT[hp][r0:r0 + 64, ki * 128:(ki + 1) * 128],
                            qT[hp][r0:r0 + 64, qc * 512 + soff:(qc + 1) * 512],
                            start=True, stop=not diag, skip_group_check=True)
                    if diag:
                        for hh in range(2):
                            nc.tensor.matmul(
                                ps_s[:, hh, soff:soff + 128],
                                id_sb[:], tri_sb[:],
                                start=False, stop=True, skip_group_check=True)
                    pt = pt_pool.tile([128, 2, 512], bf16, tag="pt")
                    if diag and soff:
                        for hh in range(2):
                            nc.scalar.activation(pt[:, hh, soff:512],
                                                 ps_s[:, hh, soff:512], Exp, scale=0.125)
                    else:
                        nc.scalar.activation(
                            pt[:].rearrange("p a b -> p (a b)"),
                            ps_s[:].rearrange("p a b -> p (a b)"), Exp, scale=0.125)
                    # defer AV(ki) until after scores(ki+1): PE stays busy
                    # while Act computes exp(ki)
                    if pend is not None:
                        emit_av(*pend)
                    pend = (ki, pt, soff)
                if pend is not None:
                    emit_av(*pend)
                for hh in range(2):
                    r0 = hh * 64
                    # sums arrive replicated in psum partitions 64..127, so
                    # normalize is just recip + multiply on DVE — no
                    # broadcast DMA, no eviction
                    rrec = nrm_pool.tile([64, 512], f32, tag="rrec")
                    nc.vector.reciprocal(rrec[:], ps_o2[hh][64:128, :])
                    nc.vector.tensor_tensor(
                        a_sb[hp][r0:r0 + 64, qc * 512:(qc + 1) * 512],
                        ps_o2[hh][0:64, :], rrec[:], MUL)

            def out_tile(qc, tl):
                ti = qc * 4 + tl
                psy = ps_s_pool.tile([128, 2, 512], f32, tag="s")
                for ncol in range(2):
                    for kc2 in range(2):
                        nc.tensor.matmul(psy[:, ncol, :], a_sb[kc2][:, ti * 128:(ti + 1) * 128],
                                         wo_sb[:, kc2, ncol * 512:(ncol + 1) * 512],
                                         start=(kc2 == 0), stop=(kc2 == 1),
                                         skip_group_check=True)
                yt = y_pool.tile([128, C], bf16, tag="yt")
                nc.vector.tensor_copy(yt[:], psy[:].rearrange("p a b -> p (a b)"))
                nc.sync.dma_start(y_d[ti * 128:(ti + 1) * 128, :], yt[:])

            for qtr in range(NQTR):
                era1(qtr)
                if 2 in phases:
                    # two out-projection tiles of the previous chunk after
                    # each head pair: the yt/psum round-trip hides behind a
                    # pair's worth of score matmuls
                    for hp in range(2):
                        era2_attn(qtr, hp)
                        if qtr > 0:
                            out_tile(qtr - 1, 2 * hp)
                            out_tile(qtr - 1, 2 * hp + 1)
            if 2 in phases:
                for tl in range(4):
                    out_tile(NQTR - 1, tl)

          for u in range(unroll):
              iter_body(u)

    if split_waits:
        _split_multi_waits(nc)
    return nc


# ---------------------------------------------------------------------------
# Host-side sharding / gather
# ---------------------------------------------------------------------------

def _rope_tables():
    inv_freq = (1.0 / (ROPE_BASE ** (np.arange(0, D, 2, dtype=np.float32) / D))).astype(np.float32)
    ang = (np.arange(T, dtype=np.float32)[:, None] * inv_freq[None, :]).astype(np.float32)  # (T, 32)
    cos, sin = np.cos(ang), np.sin(ang)
    idx = np.arange(128) % 32
    return np.ascontiguousarray(cos[:, idx].T), np.ascontiguousarray(sin[:, idx].T)  # (128, T)


def _perm_cols(g):
    """w_qkv column order for core group g: [QE|QO|KE|KO|V]."""
    cols = []
    for base, par in ((0, 0), (0, 1), (C, 0), (C, 1)):      # QE, QO, KE, KO
        for hl in range(4):
            hg = g * 4 + hl
            for i in range(32):
                cols.append(base + hg * 64 + 2 * i + par)
    for hl in range(4):
        hg = g * 4 + hl
        for d_ in range(64):
            cols.append(2 * C + hg * 64 + d_)
    return np.asarray(cols)


def make_in_maps(x, w_qkv, w_out):
    import ml_dtypes
    bf16 = ml_dtypes.bfloat16
    x = np.asarray(x, np.float32)
    w_qkv = np.asarray(w_qkv, np.float32)
    w_out = np.asarray(w_out, np.float32)
    cs, sn = _rope_tables()
    # additive causal mask for S^T[k, q] diagonal blocks: 0 iff q >= k
    tri = np.where(np.tril(np.ones((128, 128), np.float32)).T > 0, 0.0, -240.0
                   ).astype(np.float32)
    ident = np.eye(128, dtype=np.float32)
    in_maps = []
    for c in range(NCORES):
        b, g = c // 4, c % 4
        in_maps.append({
            "x": np.ascontiguousarray(x[b].T).astype(bf16),
            "w": np.ascontiguousarray(w_qkv[:, _perm_cols(g)]).astype(bf16),
            "wo": np.ascontiguousarray(w_out[g * 256:(g + 1) * 256, :]).astype(bf16),
            "cs": cs.astype(bf16), "sn": sn.astype(bf16),
            "tri": tri.astype(bf16), "id": ident.astype(bf16),
        })
    return in_maps


def kernel(x, w_qkv, w_out):
    global _BUILT
    from concourse.bass_utils import run_bass_kernel_spmd

    if _BUILT is None:
        _BUILT = build_nc()
    in_maps = make_in_maps(x, w_qkv, w_out)
    res = run_bass_kernel_spmd(_BUILT, in_maps, core_ids=list(range(NCORES)))
    out = np.zeros((B, T, C), np.float32)
    for c in range(NCORES):
        out[c // 4] += res.results[c]["y"].astype(np.float32)
    return out

